# revision 18
# baseline (speedup 1.0000x reference)
"""Trainium2 Bass kernel for nn_Net_69114613727316 (RGCN message passing).

Self-contained: kernel(**inputs) -> np.ndarray [100000] float32.

Math (exploiting num_bases=1): w[r] = att[r,0] * basis, so
    agg_d = ((sum_e v_e * x[src_e]) @ basis),  v_e = att_l[etype_e]/max(cnt_d,1)
    out = relu(agg + x @ root + bias)
The per-edge GEMMs collapse into a weighted scatter-add (one-hot matmuls
into PSUM per 256-dst "superband" window) plus one dense GEMM per window.

Sharding: node space is permuted so core c owns rows
[c*25088, (c+1)*25088) = [12544 var slots | 12544 con slots].  Each
layer's full x (bf16) is replicated via AllGather so any core can gather
arbitrary src rows; each core computes only its own rows.

v5:
  * bf16 datapath everywhere (fp32 matmul = 4 PE passes on TRN2)
  * edge-row gathers via gpsimd.dma_gather (mlp ucode), one call per
    (band-group, src-chunk), int16 chunk-local indices, spread over 4
    SWDGE queues (queues generate descriptors concurrently)
  * v_e is factored as att_l[etype] * invc[dst]: the layer-INDEPENDENT
    one-hot weight matrix o_common = onehot(dl) * invc is built on
    device ONCE (DVE), stored to DRAM, and re-loaded per layer; per tile
    only a cheap per-edge att row-scale remains (alternating DVE/ACT)
  * iota/identity shipped as inputs; IO batched per group
"""
import numpy as np
import ml_dtypes

import concourse.bass as bass
import concourse.bacc as bacc
import concourse.tile as tile
import concourse.mybir as mybir
from concourse import library_config

F32 = mybir.dt.float32
BF16 = mybir.dt.bfloat16
I16 = mybir.dt.int16
D = 128
P = 128
SB = 256                      # superband width (dst window)

N_VAR = 100000
N_CON = 100000
N_EDGES = 640000
N_CORES = 8
GSB = 12                      # superbands per group

NV_CORE = N_VAR // N_CORES          # 12500
NC_CORE = N_CON // N_CORES          # 12500
NV_SLOT = ((NV_CORE + P - 1) // P) * P   # 12544
NC_SLOT = ((NC_CORE + P - 1) // P) * P   # 12544
PER_CORE = NV_SLOT + NC_SLOT        # 25088
N_PAD = N_CORES * PER_CORE          # 200704
BANDS = PER_CORE // P               # 196
NSB = PER_CORE // SB                # 98 superbands

BF = ml_dtypes.bfloat16


def _bf(a):
    return np.asarray(a, np.float32).astype(BF)


def _preprocess(inputs):
    vf = np.ascontiguousarray(np.asarray(inputs["var_node_features"], np.float32))
    cf = np.ascontiguousarray(np.asarray(inputs["con_node_features"], np.float32))
    ei = np.asarray(inputs["edge_index"])
    et = np.asarray(inputs["edge_types"]).astype(np.int64)
    assert (np.asarray(inputs["assoc_var"]) == np.arange(N_VAR)).all()
    assert (np.asarray(inputs["assoc_con"]) == N_VAR + np.arange(N_CON)).all()

    src = ei[0].astype(np.int64)
    dst = ei[1].astype(np.int64)

    def pi(node):
        isv = node < N_VAR
        k = np.where(isv, node, node - N_VAR)
        cdiv = np.where(isv, NV_CORE, NC_CORE)
        return (k // cdiv) * PER_CORE + np.where(isv, 0, NV_SLOT) + k % cdiv

    psrc = pi(src)
    pdst = pi(dst)
    cnt = np.bincount(pdst, minlength=N_PAD).astype(np.float32)
    invc = 1.0 / np.maximum(cnt, 1.0)
    atts = [np.asarray(inputs[f"att{l}"], np.float32)[:, 0] for l in (1, 2, 3)]

    core = pdst // PER_CORE
    sb = (pdst % PER_CORE) // SB          # superband 0..97
    dl2 = (pdst % SB).astype(np.float32)  # dst offset in window, 0..255
    chunk = psrc // PER_CORE              # src chunk 0..7
    lsrc = (psrc % PER_CORE).astype(np.int16)  # chunk-local row, < 25088

    # tiles per (superband, chunk): shared across cores (max)
    seg = (core * NSB + sb) * N_CORES + chunk
    counts = np.bincount(seg, minlength=N_CORES * NSB * N_CORES).reshape(
        N_CORES, NSB, N_CORES)
    tpb2 = np.ceil(counts.max(axis=0) / P).astype(np.int64)  # [NSB, CHUNKS]

    # canonical tile-column order: group g -> chunk c -> sb in g -> tiles
    groups = [list(range(g0, min(g0 + GSB, NSB))) for g0 in range(0, NSB, GSB)]
    colmap = {}
    ncol = 0
    for g, sbs in enumerate(groups):
        for c in range(N_CORES):
            for s in sbs:
                colmap[(s, c)] = ncol
                ncol += int(tpb2[s, c])
    T2 = ncol

    # rank of each edge within its (core, sb, chunk) run
    order = np.argsort(seg, kind="stable")
    seg_s = seg[order]
    run_starts = np.concatenate(
        [[0], np.cumsum(np.bincount(seg_s, minlength=seg.max() + 1))[:-1]])
    rank = np.arange(len(order)) - run_starts[seg_s]
    core_s = core[order]
    col0 = np.array([colmap[(s, c)] for s, c in
                     zip(sb[order].tolist(), chunk[order].tolist())])
    tcol = col0 + rank // P
    prow = rank % P

    lsrc_arr = np.zeros((N_CORES, P, T2), np.int16)
    dl_arr = np.zeros((N_CORES, P, T2), np.float32)
    vw_arr = np.zeros((3, N_CORES, P, T2), np.float32)
    lsrc_arr[core_s, prow, tcol] = lsrc[order]
    dl_arr[core_s, prow, tcol] = dl2[order]
    for li in range(3):
        vw_arr[li, core_s, prow, tcol] = atts[li][et[order]] * invc[pdst[order]]

    # int16 idxs in dma_gather layout: for linear j in a call, value at
    # partition 16*r + j%16 (replicated r=0..7), column tcol*8 + prow//16.
    idx16 = np.zeros((N_CORES, P, T2 * 8), np.int16)
    tc_all = np.repeat(np.arange(T2), P)
    pr_all = np.tile(np.arange(P), T2)
    cols = tc_all * 8 + pr_all // 16
    rows = pr_all % 16
    for cidx in range(N_CORES):
        vals = lsrc_arr[cidx, pr_all, tc_all]
        for r in range(8):
            idx16[cidx, 16 * r + rows, cols] = vals

    # per-(group,chunk) call shapes
    call_tiles = []
    for g, sbs in enumerate(groups):
        call_tiles.append(
            [int(sum(int(tpb2[s, c]) for s in sbs)) for c in range(N_CORES)])

    iota256 = np.broadcast_to(
        np.arange(SB, dtype=np.float32), (P, SB)).astype(BF)
    ident = np.eye(P, dtype=np.float32).astype(BF)

    in_maps = []
    for c in range(N_CORES):
        vfeat = np.zeros((NV_SLOT, 2), np.float32)
        vfeat[:NV_CORE] = vf[c * NV_CORE:(c + 1) * NV_CORE]
        cfeat = np.zeros((NC_SLOT, 2), np.float32)
        cfeat[:NC_CORE] = cf[c * NC_CORE:(c + 1) * NC_CORE]
        m = {
            "vfeatT": _bf(vfeat.T),
            "cfeatT": _bf(cfeat.T),
            "idx16": idx16[c],
            "dl2": dl_arr[c],
            "vw1": vw_arr[0, c],
            "vw2": vw_arr[1, c],
            "vw3": vw_arr[2, c],
            "iota256": iota256,
            "ident": ident,
            "fc1_w": _bf(inputs["fc1_w"]),
            "fc1_b": np.asarray(inputs["fc1_b"], np.float32),
            "fc4_w": _bf(inputs["fc4_w"]),
            "fc4_b": np.broadcast_to(
                np.asarray(inputs["fc4_b"], np.float32).reshape(1, 1),
                (P, 1)).copy(),
        }
        for t in ("var", "con"):
            m[f"{t}_w1"] = _bf(inputs[f"{t}_w1"])
            m[f"{t}_b1"] = np.asarray(inputs[f"{t}_b1"], np.float32)
            m[f"{t}_w2"] = _bf(inputs[f"{t}_w2"])
            m[f"{t}_b2"] = np.asarray(inputs[f"{t}_b2"], np.float32)
        for l in (1, 2, 3):
            m[f"basis{l}"] = _bf(np.asarray(inputs[f"basis{l}"],
                                            np.float32).reshape(D, D))
            m[f"root{l}"] = _bf(inputs[f"root{l}"])
            m[f"bias{l}"] = np.asarray(inputs[f"bias{l}"], np.float32)
        in_maps.append(m)
    key = tuple(int(t) for t in tpb2.reshape(-1))
    return in_maps, tpb2, groups, colmap, call_tiles, T2, key


def _build_program(tpb2, groups, colmap, call_tiles, T2):
    nc = bacc.Bacc("TRN2", target_bir_lowering=False, debug=False,
                   num_devices=N_CORES, num_swdge_queues=4)

    def inp(name, shape, dtype=BF16):
        return nc.dram_tensor(name, shape, dtype, kind="ExternalInput")

    vfeatT = inp("vfeatT", [2, NV_SLOT])
    cfeatT = inp("cfeatT", [2, NC_SLOT])
    idx16 = inp("idx16", [P, T2 * 8], I16)
    dl2 = inp("dl2", [P, T2], F32)
    vws = {l: inp(f"vw{l}", [P, T2], F32) for l in (1, 2, 3)}
    iota256 = inp("iota256", [P, SB])
    identt = inp("ident", [P, P])
    mlp_w = {}
    for t in ("var", "con"):
        mlp_w[t] = (inp(f"{t}_w1", [2, D]), inp(f"{t}_b1", [D], F32),
                    inp(f"{t}_w2", [D, D]), inp(f"{t}_b2", [D], F32))
    rg_w = {l: (inp(f"basis{l}", [D, D]), inp(f"root{l}", [D, D]),
                inp(f"bias{l}", [D], F32)) for l in (1, 2, 3)}
    fc1_w = inp("fc1_w", [4 * D, D])
    fc1_b = inp("fc1_b", [D], F32)
    fc4_w = inp("fc4_w", [D, 1])
    fc4_b = inp("fc4_b", [P, 1], F32)
    y_out = nc.dram_tensor("y_out", [NV_SLOT], F32, kind="ExternalOutput")

    x_full = [nc.dram_tensor(f"x{i}_full", [N_PAD, D], BF16, kind="Internal",
                             addr_space="Shared") for i in range(3)]
    ag_in = [nc.dram_tensor(f"ag_in{i}", [PER_CORE, D], BF16, kind="Internal")
             for i in range(3)]
    xT_own = [nc.dram_tensor(f"xT{i}_own", [D, PER_CORE], BF16,
                             kind="Internal") for i in range(4)]

    rgroups = [list(range(N_CORES))]
    grp_cols = []
    grp_cstart = []
    grp_base = []
    base = 0
    for g, sbs in enumerate(groups):
        cst = []
        off = 0
        for c in range(N_CORES):
            cst.append(off)
            off += call_tiles[g][c]
        grp_cstart.append(cst)
        grp_cols.append(off)
        grp_base.append(base)
        base += off
    max_grp_cols = max(grp_cols)

    with tile.TileContext(nc) as tc:
        with tc.tile_pool(name="wp", bufs=1) as wp, \
             tc.tile_pool(name="pre", bufs=34) as prep:
            nc.gpsimd.load_library(library_config.mlp)
            iota_t = wp.tile([P, SB], BF16, name="iota_t")
            nc.sync.dma_start(iota_t[:], iota256[:])
            ident = wp.tile([P, P], BF16, name="ident")
            nc.sync.dma_start(ident[:], identt[:])

            idx_sb = wp.tile([P, T2 * 8], I16, name="idx_sb")
            nc.sync.dma_start(idx_sb[:], idx16[:])
            dl_sb = wp.tile([P, T2], F32, name="dl_sb")
            nc.sync.dma_start(dl_sb[:], dl2[:])
            vw_sb = {}
            for l in (1, 2, 3):
                vw_sb[l] = wp.tile([P, T2], F32, name=f"vw_sb{l}")
                nc.sync.dma_start(vw_sb[l][:], vws[l][:])

            # ---------- phase A: input MLPs -> x0 ----------
            GB_A = 8
            with tc.tile_pool(name="pa_f", bufs=1) as fp, \
                 tc.tile_pool(name="pa_sb", bufs=3) as sp, \
                 tc.tile_pool(name="pa_o", bufs=2) as op_a, \
                 tc.tile_pool(name="pa_pst", bufs=2, space="PSUM") as pp_t, \
                 tc.tile_pool(name="pa_ps", bufs=2, space="PSUM") as pp:
                for ttype, featT, slot0, nslot in (
                    ("var", vfeatT, 0, NV_SLOT),
                    ("con", cfeatT, NV_SLOT, NC_SLOT),
                ):
                    w1, b1, w2, b2 = mlp_w[ttype]
                    w1s = wp.tile([2, D], BF16, name=f"w1s_{ttype}")
                    nc.sync.dma_start(w1s[:], w1[:])
                    b1s = wp.tile([P, 1], F32, name=f"b1s_{ttype}")
                    nc.sync.dma_start(b1s[:], b1.rearrange("(p one) -> p one", one=1))
                    w2s = wp.tile([D, D], BF16, name=f"w2s_{ttype}")
                    nc.sync.dma_start(w2s[:], w2[:])
                    b2s = wp.tile([P, 1], F32, name=f"b2s_{ttype}")
                    nc.sync.dma_start(b2s[:], b2.rearrange("(p one) -> p one", one=1))
                    ftT = fp.tile([2, nslot], BF16, name=f"ftT_{ttype}")
                    nc.sync.dma_start(ftT[:], featT[:])
                    nbands_t = nslot // P
                    for jg0 in range(0, nbands_t, GB_A):
                        jgn = min(GB_A, nbands_t - jg0)
                        xTg = op_a.tile([P, GB_A * P], BF16, name="xTg")
                        rowg = op_a.tile([P, GB_A * P], BF16, name="rowg")
                        for jj in range(jgn):
                            j = jg0 + jj
                            p1 = pp.tile([P, P], F32, name="p1", space="PSUM")
                            nc.tensor.matmul(p1[:], lhsT=w1s[:],
                                             rhs=ftT[:, j * P:(j + 1) * P],
                                             start=True, stop=True)
                            h1 = sp.tile([P, P], BF16, name="h1")
                            nc.scalar.activation(h1[:], p1[:],
                                                 mybir.ActivationFunctionType.Relu,
                                                 bias=b1s[:, :1])
                            p2 = pp.tile([P, P], F32, name="p2", space="PSUM")
                            nc.tensor.matmul(p2[:], lhsT=w2s[:], rhs=h1[:],
                                             start=True, stop=True)
                            nc.vector.tensor_scalar(
                                xTg[:, jj * P:(jj + 1) * P], p2[:],
                                b2s[:, :1], None, op0=mybir.AluOpType.add)
                            tp = pp_t.tile([P, P], BF16, name="tp", space="PSUM")
                            nc.tensor.transpose(
                                tp[:], xTg[:, jj * P:(jj + 1) * P], ident[:])
                            nc.scalar.activation(
                                rowg[:, jj * P:(jj + 1) * P], tp[:],
                                mybir.ActivationFunctionType.Copy)
                        col0 = slot0 + jg0 * P
                        ncols = jgn * P
                        nc.sync.dma_start(
                            xT_own[0][:, col0:col0 + ncols], xTg[:, :ncols])
                        nc.sync.dma_start(
                            ag_in[0][col0:col0 + ncols, :].rearrange(
                                "(t p) d -> p t d", p=P),
                            rowg[:, :ncols].rearrange(
                                "p (t d) -> p t d", d=D))
            opre_tiles = {}

            def emit_opre(lnext):
                tiles = []
                for col in range(min(32, grp_cols[0])):
                    ot = prep.tile([P, SB], BF16, name="opre")
                    nc.vector.tensor_scalar(
                        ot[:], iota_t[:],
                        dl_sb[:, col:col + 1],
                        vw_sb[lnext][:, col:col + 1],
                        op0=mybir.AluOpType.is_equal,
                        op1=mybir.AluOpType.mult)
                    tiles.append(ot)
                opre_tiles[lnext] = tiles

            emit_opre(1)
            nc.gpsimd.collective_compute(
                "AllGather", mybir.AluOpType.bypass, replica_groups=rgroups,
                ins=[ag_in[0][:]], outs=[x_full[0][:]])

            # head-MLP weights (consumed inside layer 3)
            fc1c = []
            for hl in range(4):
                t = wp.tile([D, D], BF16, name=f"fc1c{hl}")
                nc.sync.dma_start(t[:], fc1_w[hl * D:(hl + 1) * D, :])
                fc1c.append(t)
            fb1 = wp.tile([P, 1], F32, name="fb1")
            nc.sync.dma_start(fb1[:], fc1_b.rearrange("(p one) -> p one", one=1))
            f4w = wp.tile([D, 1], BF16, name="f4w")
            nc.sync.dma_start(f4w[:], fc4_w[:])
            f4b = wp.tile([P, 1], F32, name="f4b")
            nc.sync.dma_start(f4b[:], fc4_b[:])
            NSB_VAR = NV_SLOT // SB   # var superbands: 0..48

            # ---------- phases B: 3 RGCN layers ----------
            for l in (1, 2, 3):
                basis, root, bias = rg_w[l]
                Bs = wp.tile([D, D], BF16, name=f"Bs_{l}")
                nc.sync.dma_start(Bs[:], basis[:])
                Rs = wp.tile([D, D], BF16, name=f"Rs_{l}")
                nc.sync.dma_start(Rs[:], root[:])
                bs = wp.tile([P, 1], F32, name=f"bs_{l}")
                nc.sync.dma_start(bs[:], bias.rearrange("(p one) -> p one", one=1))
                xcur = x_full[l - 1]
                xTc = xT_own[l - 1]
                xTn = xT_own[l]
                vwl = vw_sb[l]
                with tc.tile_pool(name=f"l{l}_g", bufs=2) as gp, \
                     tc.tile_pool(name=f"l{l}_sb", bufs=3) as sp, \
                     tc.tile_pool(name=f"l{l}_og", bufs=2) as og, \
                     tc.tile_pool(name=f"l{l}_o", bufs=32) as op, \
                     tc.tile_pool(name=f"l{l}_ps", bufs=2, space="PSUM") as pp, \
                     tc.tile_pool(name=f"l{l}_pt", bufs=2, space="PSUM") as pt, \
                     tc.tile_pool(name=f"l{l}_p2", bufs=2, space="PSUM") as pq, \
                     tc.tile_pool(name=f"l{l}_py", bufs=2, space="PSUM") as py:
                    for g, sbs in enumerate(groups):
                        gb = grp_base[g]
                        gn = len(sbs)
                        gcols = grp_cols[g]
                        xg = gp.tile([P, max_grp_cols * P], BF16, name="xg")
                        for c in range(N_CORES):
                            nt = call_tiles[g][c]
                            if nt == 0:
                                continue
                            cs = grp_cstart[g][c]
                            nidx = nt * P
                            nc.gpsimd.dma_gather(
                                xg[:, cs * P:(cs + nt) * P].rearrange(
                                    "p (t e) -> p t e", e=D),
                                xcur[c * PER_CORE:(c + 1) * PER_CORE, :],
                                idx_sb[:, (gb + cs) * 8:(gb + cs + nt) * 8],
                                nidx, nidx, D, single_packet=False,
                                queue_num=c % 4)
                        xtbg = og.tile([P, GSB * SB], BF16, name="xtbg")
                        nc.sync.dma_start(
                            xtbg[:, :gn * SB],
                            xTc[:, sbs[0] * SB:(sbs[0] + gn) * SB])
                        outg = og.tile([P, GSB * SB], BF16, name="outg")
                        if l < 3:
                            rowg = og.tile([P, GSB * SB], BF16, name="rowg")
                        for si, s in enumerate(sbs):
                            cols = []
                            for c in range(N_CORES):
                                c0 = grp_cstart[g][c] + sum(
                                    int(tpb2[s2, c]) for s2 in sbs[:si])
                                for t in range(int(tpb2[s, c])):
                                    cols.append(c0 + t)
                            st = pp.tile([P, SB], F32, name="st", space="PSUM")
                            for ti, col in enumerate(cols):
                                gcol = gb + col
                                if g == 0 and col < len(opre_tiles[l]):
                                    rhs_o = opre_tiles[l][col][:]
                                else:
                                    o = op.tile([P, SB], BF16, name="o")
                                    nc.vector.tensor_scalar(
                                        o[:], iota_t[:],
                                        dl_sb[:, gcol:gcol + 1],
                                        vwl[:, gcol:gcol + 1],
                                        op0=mybir.AluOpType.is_equal,
                                        op1=mybir.AluOpType.mult)
                                    rhs_o = o[:]
                                nc.tensor.matmul(
                                    st[:], lhsT=xg[:, col * P:(col + 1) * P],
                                    rhs=rhs_o,
                                    start=(ti == 0),
                                    stop=(ti == len(cols) - 1))
                            p2 = pq.tile([P, SB], F32, name="p2", space="PSUM")
                            if cols:
                                sn = sp.tile([P, SB], BF16, name="sn")
                                nc.scalar.activation(
                                    sn[:], st[:],
                                    mybir.ActivationFunctionType.Copy)
                                nc.tensor.matmul(p2[:], lhsT=Bs[:], rhs=sn[:],
                                                 start=True, stop=False)
                                nc.tensor.matmul(
                                    p2[:], lhsT=Rs[:],
                                    rhs=xtbg[:, si * SB:(si + 1) * SB],
                                    start=False, stop=True)
                            else:
                                nc.tensor.matmul(
                                    p2[:], lhsT=Rs[:],
                                    rhs=xtbg[:, si * SB:(si + 1) * SB],
                                    start=True, stop=True)
                            nc.scalar.activation(
                                outg[:, si * SB:(si + 1) * SB], p2[:],
                                mybir.ActivationFunctionType.Relu,
                                bias=bs[:, :1])
                            if l < 3:
                                for h in range(2):
                                    c0 = si * SB + h * P
                                    tp2 = pt.tile([P, P], BF16, name="tp2",
                                                  space="PSUM")
                                    nc.tensor.transpose(
                                        tp2[:], outg[:, c0:c0 + P], ident[:])
                                    nc.scalar.activation(
                                        rowg[:, c0:c0 + P], tp2[:],
                                        mybir.ActivationFunctionType.Copy)
                        nc.sync.dma_start(
                            xTn[:, sbs[0] * SB:(sbs[0] + gn) * SB],
                            outg[:, :gn * SB])
                        if l < 3:
                            nc.sync.dma_start(
                                ag_in[l][sbs[0] * SB:(sbs[0] + gn) * SB, :]
                                .rearrange("(t p) d -> p t d", p=P),
                                rowg[:, :gn * SB].rearrange(
                                    "p (t d) -> p t d", d=D))
                        if l == 3 and sbs[0] < NSB_VAR:
                            gnv = sum(1 for s in sbs if s < NSB_VAR)
                            xtls = []
                            for hl in range(3):
                                xtl = og.tile([P, GSB * SB], BF16,
                                              name=f"xtl{hl}")
                                nc.sync.dma_start(
                                    xtl[:, :gnv * SB],
                                    xT_own[hl][:, sbs[0] * SB:
                                               (sbs[0] + gnv) * SB])
                                xtls.append(xtl)
                            ybuf = og.tile([P, 2 * GSB], F32, name="ybuf")
                            for si in range(gnv):
                                for h in range(2):
                                    cc0 = si * SB + h * P
                                    hp = pt.tile([P, P], F32, name="hp",
                                                 space="PSUM")
                                    for hl in range(3):
                                        nc.tensor.matmul(
                                            hp[:], lhsT=fc1c[hl][:],
                                            rhs=xtls[hl][:, cc0:cc0 + P],
                                            start=(hl == 0), stop=False)
                                    nc.tensor.matmul(
                                        hp[:], lhsT=fc1c[3][:],
                                        rhs=outg[:, cc0:cc0 + P],
                                        start=False, stop=True)
                                    hr = sp.tile([P, P], BF16, name="hr")
                                    nc.scalar.activation(
                                        hr[:], hp[:],
                                        mybir.ActivationFunctionType.Relu,
                                        bias=fb1[:, :1])
                                    yp = py.tile([P, 1], F32, name="yp",
                                                 space="PSUM")
                                    nc.tensor.matmul(yp[:], lhsT=hr[:],
                                                     rhs=f4w[:],
                                                     start=True, stop=True)
                                    nc.vector.tensor_scalar(
                                        ybuf[:, si * 2 + h:si * 2 + h + 1],
                                        yp[:], f4b[:, :1], None,
                                        op0=mybir.AluOpType.add)
                            nc.sync.dma_start(
                                y_out[sbs[0] * SB:(sbs[0] + gnv) * SB]
                                .rearrange("(g p) -> p g", p=P),
                                ybuf[:, :gnv * 2])
                if l < 3:
                    emit_opre(l + 1)
                    nc.gpsimd.collective_compute(
                        "AllGather", mybir.AluOpType.bypass,
                        replica_groups=rgroups,
                        ins=[ag_in[l][:]], outs=[x_full[l][:]])

    nc.compile()
    return nc


_CACHE = {}


def kernel(**inputs) -> np.ndarray:
    from concourse import bass_utils
    in_maps, tpb2, groups, colmap, call_tiles, T2, key = _preprocess(inputs)
    if key not in _CACHE:
        _CACHE[key] = _build_program(tpb2, groups, colmap, call_tiles, T2)
    nc = _CACHE[key]
    res = bass_utils.run_bass_kernel_spmd(
        nc, in_maps, core_ids=list(range(N_CORES)))
    ys = [res.results[c]["y_out"][:NV_CORE] for c in range(N_CORES)]
    return np.concatenate(ys, axis=0).astype(np.float32)


# revision 20
# speedup vs baseline: 1.0101x; 1.0101x over previous
"""Trainium2 Bass kernel for nn_Net_69114613727316 (RGCN message passing).

Self-contained: kernel(**inputs) -> np.ndarray [100000] float32.

Math (exploiting num_bases=1): w[r] = att[r,0] * basis, so
    agg_d = ((sum_e v_e * x[src_e]) @ basis),  v_e = att_l[etype_e]/max(cnt_d,1)
    out = relu(agg + x @ root + bias)
The per-edge GEMMs collapse into a weighted scatter-add (one-hot matmuls
into PSUM per 256-dst "superband" window) plus one dense GEMM per window.

Sharding: node space is permuted so core c owns rows
[c*25088, (c+1)*25088) = [12544 var slots | 12544 con slots].  Each
layer's full x (bf16) is replicated via AllGather so any core can gather
arbitrary src rows; each core computes only its own rows.

v5:
  * bf16 datapath everywhere (fp32 matmul = 4 PE passes on TRN2)
  * edge-row gathers via gpsimd.dma_gather (mlp ucode), one call per
    (band-group, src-chunk), int16 chunk-local indices, spread over 4
    SWDGE queues (queues generate descriptors concurrently)
  * v_e is factored as att_l[etype] * invc[dst]: the layer-INDEPENDENT
    one-hot weight matrix o_common = onehot(dl) * invc is built on
    device ONCE (DVE), stored to DRAM, and re-loaded per layer; per tile
    only a cheap per-edge att row-scale remains (alternating DVE/ACT)
  * iota/identity shipped as inputs; IO batched per group
"""
import numpy as np
import ml_dtypes

import concourse.bass as bass
import concourse.bacc as bacc
import concourse.tile as tile
import concourse.mybir as mybir
from concourse import library_config

F32 = mybir.dt.float32
BF16 = mybir.dt.bfloat16
I16 = mybir.dt.int16
D = 128
P = 128
SB = 256                      # superband width (dst window)

N_VAR = 100000
N_CON = 100000
N_EDGES = 640000
N_CORES = 8
GSB = 12                      # superbands per group

NV_CORE = N_VAR // N_CORES          # 12500
NC_CORE = N_CON // N_CORES          # 12500
NV_SLOT = ((NV_CORE + P - 1) // P) * P   # 12544
NC_SLOT = ((NC_CORE + P - 1) // P) * P   # 12544
PER_CORE = NV_SLOT + NC_SLOT        # 25088
N_PAD = N_CORES * PER_CORE          # 200704
BANDS = PER_CORE // P               # 196
NSB = PER_CORE // SB                # 98 superbands

BF = ml_dtypes.bfloat16


def _bf(a):
    return np.asarray(a, np.float32).astype(BF)


def _preprocess(inputs):
    vf = np.ascontiguousarray(np.asarray(inputs["var_node_features"], np.float32))
    cf = np.ascontiguousarray(np.asarray(inputs["con_node_features"], np.float32))
    ei = np.asarray(inputs["edge_index"])
    et = np.asarray(inputs["edge_types"]).astype(np.int64)
    assert (np.asarray(inputs["assoc_var"]) == np.arange(N_VAR)).all()
    assert (np.asarray(inputs["assoc_con"]) == N_VAR + np.arange(N_CON)).all()

    src = ei[0].astype(np.int64)
    dst = ei[1].astype(np.int64)

    def pi(node):
        isv = node < N_VAR
        k = np.where(isv, node, node - N_VAR)
        cdiv = np.where(isv, NV_CORE, NC_CORE)
        return (k // cdiv) * PER_CORE + np.where(isv, 0, NV_SLOT) + k % cdiv

    psrc = pi(src)
    pdst = pi(dst)
    cnt = np.bincount(pdst, minlength=N_PAD).astype(np.float32)
    invc = 1.0 / np.maximum(cnt, 1.0)
    atts = [np.asarray(inputs[f"att{l}"], np.float32)[:, 0] for l in (1, 2, 3)]

    core = pdst // PER_CORE
    sb = (pdst % PER_CORE) // SB          # superband 0..97
    dl2 = (pdst % SB).astype(np.float32)  # dst offset in window, 0..255
    chunk = psrc // PER_CORE              # src chunk 0..7
    lsrc = (psrc % PER_CORE).astype(np.int16)  # chunk-local row, < 25088

    # tiles per (superband, chunk): shared across cores (max)
    seg = (core * NSB + sb) * N_CORES + chunk
    counts = np.bincount(seg, minlength=N_CORES * NSB * N_CORES).reshape(
        N_CORES, NSB, N_CORES)
    tpb2 = np.ceil(counts.max(axis=0) / P).astype(np.int64)  # [NSB, CHUNKS]

    # canonical tile-column order: group g -> chunk c -> sb in g -> tiles
    groups = [list(range(g0, min(g0 + GSB, NSB))) for g0 in range(0, NSB, GSB)]
    colmap = {}
    ncol = 0
    for g, sbs in enumerate(groups):
        for c in range(N_CORES):
            for s in sbs:
                colmap[(s, c)] = ncol
                ncol += int(tpb2[s, c])
    T2 = ncol

    # rank of each edge within its (core, sb, chunk) run
    order = np.argsort(seg, kind="stable")
    seg_s = seg[order]
    run_starts = np.concatenate(
        [[0], np.cumsum(np.bincount(seg_s, minlength=seg.max() + 1))[:-1]])
    rank = np.arange(len(order)) - run_starts[seg_s]
    core_s = core[order]
    col0 = np.array([colmap[(s, c)] for s, c in
                     zip(sb[order].tolist(), chunk[order].tolist())])
    tcol = col0 + rank // P
    prow = rank % P

    lsrc_arr = np.zeros((N_CORES, P, T2), np.int16)
    dl_arr = np.zeros((N_CORES, P, T2), np.float32)
    vw_arr = np.zeros((3, N_CORES, P, T2), np.float32)
    lsrc_arr[core_s, prow, tcol] = lsrc[order]
    dl_arr[core_s, prow, tcol] = dl2[order]
    for li in range(3):
        vw_arr[li, core_s, prow, tcol] = atts[li][et[order]] * invc[pdst[order]]

    # int16 idxs in dma_gather layout: for linear j in a call, value at
    # partition 16*r + j%16 (replicated r=0..7), column tcol*8 + prow//16.
    idx16 = np.zeros((N_CORES, P, T2 * 8), np.int16)
    tc_all = np.repeat(np.arange(T2), P)
    pr_all = np.tile(np.arange(P), T2)
    cols = tc_all * 8 + pr_all // 16
    rows = pr_all % 16
    for cidx in range(N_CORES):
        vals = lsrc_arr[cidx, pr_all, tc_all]
        for r in range(8):
            idx16[cidx, 16 * r + rows, cols] = vals

    # per-(group,chunk) call shapes
    call_tiles = []
    for g, sbs in enumerate(groups):
        call_tiles.append(
            [int(sum(int(tpb2[s, c]) for s in sbs)) for c in range(N_CORES)])

    iota256 = np.broadcast_to(
        np.arange(SB, dtype=np.float32), (P, SB)).astype(BF)
    ident = np.eye(P, dtype=np.float32).astype(BF)

    in_maps = []
    for c in range(N_CORES):
        vfeat = np.zeros((NV_SLOT, 2), np.float32)
        vfeat[:NV_CORE] = vf[c * NV_CORE:(c + 1) * NV_CORE]
        cfeat = np.zeros((NC_SLOT, 2), np.float32)
        cfeat[:NC_CORE] = cf[c * NC_CORE:(c + 1) * NC_CORE]
        m = {
            "vfeatT": _bf(vfeat.T),
            "cfeatT": _bf(cfeat.T),
            "idx16": idx16[c],
            "dl2": dl_arr[c],
            "vw1": vw_arr[0, c],
            "vw2": vw_arr[1, c],
            "vw3": vw_arr[2, c],
            "iota256": iota256,
            "ident": ident,
            "fc1_w": _bf(inputs["fc1_w"]),
            "fc1_b": np.asarray(inputs["fc1_b"], np.float32),
            "fc4_w": _bf(inputs["fc4_w"]),
            "fc4_b": np.broadcast_to(
                np.asarray(inputs["fc4_b"], np.float32).reshape(1, 1),
                (P, 1)).copy(),
        }
        for t in ("var", "con"):
            m[f"{t}_w1"] = _bf(inputs[f"{t}_w1"])
            m[f"{t}_b1"] = np.asarray(inputs[f"{t}_b1"], np.float32)
            m[f"{t}_w2"] = _bf(inputs[f"{t}_w2"])
            m[f"{t}_b2"] = np.asarray(inputs[f"{t}_b2"], np.float32)
        for l in (1, 2, 3):
            m[f"basis{l}"] = _bf(np.asarray(inputs[f"basis{l}"],
                                            np.float32).reshape(D, D))
            m[f"root{l}"] = _bf(inputs[f"root{l}"])
            m[f"bias{l}"] = np.asarray(inputs[f"bias{l}"], np.float32)
        in_maps.append(m)
    key = tuple(int(t) for t in tpb2.reshape(-1))
    return in_maps, tpb2, groups, colmap, call_tiles, T2, key


def _build_program(tpb2, groups, colmap, call_tiles, T2):
    nc = bacc.Bacc("TRN2", target_bir_lowering=False, debug=False,
                   num_devices=N_CORES, num_swdge_queues=4)

    def inp(name, shape, dtype=BF16):
        return nc.dram_tensor(name, shape, dtype, kind="ExternalInput")

    vfeatT = inp("vfeatT", [2, NV_SLOT])
    cfeatT = inp("cfeatT", [2, NC_SLOT])
    idx16 = inp("idx16", [P, T2 * 8], I16)
    dl2 = inp("dl2", [P, T2], F32)
    vws = {l: inp(f"vw{l}", [P, T2], F32) for l in (1, 2, 3)}
    iota256 = inp("iota256", [P, SB])
    identt = inp("ident", [P, P])
    mlp_w = {}
    for t in ("var", "con"):
        mlp_w[t] = (inp(f"{t}_w1", [2, D]), inp(f"{t}_b1", [D], F32),
                    inp(f"{t}_w2", [D, D]), inp(f"{t}_b2", [D], F32))
    rg_w = {l: (inp(f"basis{l}", [D, D]), inp(f"root{l}", [D, D]),
                inp(f"bias{l}", [D], F32)) for l in (1, 2, 3)}
    fc1_w = inp("fc1_w", [4 * D, D])
    fc1_b = inp("fc1_b", [D], F32)
    fc4_w = inp("fc4_w", [D, 1])
    fc4_b = inp("fc4_b", [P, 1], F32)
    y_out = nc.dram_tensor("y_out", [NV_SLOT], F32, kind="ExternalOutput")

    x_full = [nc.dram_tensor(f"x{i}_full", [N_PAD, D], BF16, kind="Internal",
                             addr_space="Shared") for i in range(3)]
    ag_in = [nc.dram_tensor(f"ag_in{i}", [PER_CORE, D], BF16, kind="Internal")
             for i in range(3)]
    xT_own = [nc.dram_tensor(f"xT{i}_own", [D, PER_CORE], BF16,
                             kind="Internal") for i in range(4)]

    rgroups = [list(range(N_CORES))]
    grp_cols = []
    grp_cstart = []
    grp_base = []
    base = 0
    for g, sbs in enumerate(groups):
        cst = []
        off = 0
        for c in range(N_CORES):
            cst.append(off)
            off += call_tiles[g][c]
        grp_cstart.append(cst)
        grp_cols.append(off)
        grp_base.append(base)
        base += off
    max_grp_cols = max(grp_cols)

    with tile.TileContext(nc) as tc:
        with tc.tile_pool(name="wp", bufs=1) as wp:
            nc.gpsimd.load_library(library_config.mlp)
            iota_t = wp.tile([P, SB], BF16, name="iota_t")
            nc.sync.dma_start(iota_t[:], iota256[:])
            ident = wp.tile([P, P], BF16, name="ident")
            nc.sync.dma_start(ident[:], identt[:])

            idx_sb = wp.tile([P, T2 * 8], I16, name="idx_sb")
            nc.sync.dma_start(idx_sb[:], idx16[:])
            dl_sb = wp.tile([P, T2], F32, name="dl_sb")
            nc.sync.dma_start(dl_sb[:], dl2[:])
            vw_sb = {}
            for l in (1, 2, 3):
                vw_sb[l] = wp.tile([P, T2], F32, name=f"vw_sb{l}")
                nc.sync.dma_start(vw_sb[l][:], vws[l][:])

            # ---------- phase A: input MLPs -> x0 ----------
            GB_A = 8
            with tc.tile_pool(name="pa_f", bufs=1) as fp, \
                 tc.tile_pool(name="pa_sb", bufs=3) as sp, \
                 tc.tile_pool(name="pa_o", bufs=2) as op_a, \
                 tc.tile_pool(name="pa_pst", bufs=2, space="PSUM") as pp_t, \
                 tc.tile_pool(name="pa_ps", bufs=2, space="PSUM") as pp:
                for ttype, featT, slot0, nslot in (
                    ("var", vfeatT, 0, NV_SLOT),
                    ("con", cfeatT, NV_SLOT, NC_SLOT),
                ):
                    w1, b1, w2, b2 = mlp_w[ttype]
                    w1s = wp.tile([2, D], BF16, name=f"w1s_{ttype}")
                    nc.sync.dma_start(w1s[:], w1[:])
                    b1s = wp.tile([P, 1], F32, name=f"b1s_{ttype}")
                    nc.sync.dma_start(b1s[:], b1.rearrange("(p one) -> p one", one=1))
                    w2s = wp.tile([D, D], BF16, name=f"w2s_{ttype}")
                    nc.sync.dma_start(w2s[:], w2[:])
                    b2s = wp.tile([P, 1], F32, name=f"b2s_{ttype}")
                    nc.sync.dma_start(b2s[:], b2.rearrange("(p one) -> p one", one=1))
                    ftT = fp.tile([2, nslot], BF16, name=f"ftT_{ttype}")
                    nc.sync.dma_start(ftT[:], featT[:])
                    nbands_t = nslot // P
                    for jg0 in range(0, nbands_t, GB_A):
                        jgn = min(GB_A, nbands_t - jg0)
                        xTg = op_a.tile([P, GB_A * P], BF16, name="xTg")
                        rowg = op_a.tile([P, GB_A * P], BF16, name="rowg")
                        for jj in range(jgn):
                            j = jg0 + jj
                            p1 = pp.tile([P, P], F32, name="p1", space="PSUM")
                            nc.tensor.matmul(p1[:], lhsT=w1s[:],
                                             rhs=ftT[:, j * P:(j + 1) * P],
                                             start=True, stop=True)
                            h1 = sp.tile([P, P], BF16, name="h1")
                            nc.scalar.activation(h1[:], p1[:],
                                                 mybir.ActivationFunctionType.Relu,
                                                 bias=b1s[:, :1])
                            p2 = pp.tile([P, P], F32, name="p2", space="PSUM")
                            nc.tensor.matmul(p2[:], lhsT=w2s[:], rhs=h1[:],
                                             start=True, stop=True)
                            nc.vector.tensor_scalar(
                                xTg[:, jj * P:(jj + 1) * P], p2[:],
                                b2s[:, :1], None, op0=mybir.AluOpType.add)
                            tp = pp_t.tile([P, P], BF16, name="tp", space="PSUM")
                            nc.tensor.transpose(
                                tp[:], xTg[:, jj * P:(jj + 1) * P], ident[:])
                            nc.scalar.activation(
                                rowg[:, jj * P:(jj + 1) * P], tp[:],
                                mybir.ActivationFunctionType.Copy)
                        col0 = slot0 + jg0 * P
                        ncols = jgn * P
                        nc.sync.dma_start(
                            xT_own[0][:, col0:col0 + ncols], xTg[:, :ncols])
                        nc.sync.dma_start(
                            ag_in[0][col0:col0 + ncols, :].rearrange(
                                "(t p) d -> p t d", p=P),
                            rowg[:, :ncols].rearrange(
                                "p (t d) -> p t d", d=D))
            nc.gpsimd.collective_compute(
                "AllGather", mybir.AluOpType.bypass, replica_groups=rgroups,
                ins=[ag_in[0][:]], outs=[x_full[0][:]])

            # head-MLP weights (consumed inside layer 3)
            fc1c = []
            for hl in range(4):
                t = wp.tile([D, D], BF16, name=f"fc1c{hl}")
                nc.sync.dma_start(t[:], fc1_w[hl * D:(hl + 1) * D, :])
                fc1c.append(t)
            fb1 = wp.tile([P, 1], F32, name="fb1")
            nc.sync.dma_start(fb1[:], fc1_b.rearrange("(p one) -> p one", one=1))
            f4w = wp.tile([D, 1], BF16, name="f4w")
            nc.sync.dma_start(f4w[:], fc4_w[:])
            f4b = wp.tile([P, 1], F32, name="f4b")
            nc.sync.dma_start(f4b[:], fc4_b[:])
            NSB_VAR = NV_SLOT // SB   # var superbands: 0..48

            # ---------- phases B: 3 RGCN layers ----------
            for l in (1, 2, 3):
                basis, root, bias = rg_w[l]
                Bs = wp.tile([D, D], BF16, name=f"Bs_{l}")
                nc.sync.dma_start(Bs[:], basis[:])
                Rs = wp.tile([D, D], BF16, name=f"Rs_{l}")
                nc.sync.dma_start(Rs[:], root[:])
                bs = wp.tile([P, 1], F32, name=f"bs_{l}")
                nc.sync.dma_start(bs[:], bias.rearrange("(p one) -> p one", one=1))
                xcur = x_full[l - 1]
                xTc = xT_own[l - 1]
                xTn = xT_own[l]
                vwl = vw_sb[l]
                with tc.tile_pool(name=f"l{l}_g", bufs=2) as gp, \
                     tc.tile_pool(name=f"l{l}_sb", bufs=3) as sp, \
                     tc.tile_pool(name=f"l{l}_og", bufs=(3 if l < 3 else 2)) as og, \
                     tc.tile_pool(name=f"l{l}_o", bufs=64) as op, \
                     tc.tile_pool(name=f"l{l}_ps", bufs=2, space="PSUM") as pp, \
                     tc.tile_pool(name=f"l{l}_pt", bufs=2, space="PSUM") as pt, \
                     tc.tile_pool(name=f"l{l}_p2", bufs=2, space="PSUM") as pq, \
                     tc.tile_pool(name=f"l{l}_py", bufs=2, space="PSUM") as py:
                    for g, sbs in enumerate(groups):
                        gb = grp_base[g]
                        gn = len(sbs)
                        gcols = grp_cols[g]
                        xg = gp.tile([P, max_grp_cols * P], BF16, name="xg")
                        for c in range(N_CORES):
                            nt = call_tiles[g][c]
                            if nt == 0:
                                continue
                            cs = grp_cstart[g][c]
                            nidx = nt * P
                            nc.gpsimd.dma_gather(
                                xg[:, cs * P:(cs + nt) * P].rearrange(
                                    "p (t e) -> p t e", e=D),
                                xcur[c * PER_CORE:(c + 1) * PER_CORE, :],
                                idx_sb[:, (gb + cs) * 8:(gb + cs + nt) * 8],
                                nidx, nidx, D, single_packet=False,
                                queue_num=c % 4)
                        xtbg = og.tile([P, GSB * SB], BF16, name="xtbg")
                        nc.sync.dma_start(
                            xtbg[:, :gn * SB],
                            xTc[:, sbs[0] * SB:(sbs[0] + gn) * SB])
                        outg = og.tile([P, GSB * SB], BF16, name="outg")
                        if l < 3:
                            rowg = og.tile([P, GSB * SB], BF16, name="rowg")
                        for si, s in enumerate(sbs):
                            cols = []
                            for c in range(N_CORES):
                                c0 = grp_cstart[g][c] + sum(
                                    int(tpb2[s2, c]) for s2 in sbs[:si])
                                for t in range(int(tpb2[s, c])):
                                    cols.append(c0 + t)
                            st = pp.tile([P, SB], F32, name="st", space="PSUM")
                            for ti, col in enumerate(cols):
                                gcol = gb + col
                                o = op.tile([P, SB], BF16, name="o")
                                nc.vector.tensor_scalar(
                                    o[:], iota_t[:],
                                    dl_sb[:, gcol:gcol + 1],
                                    vwl[:, gcol:gcol + 1],
                                    op0=mybir.AluOpType.is_equal,
                                    op1=mybir.AluOpType.mult)
                                nc.tensor.matmul(
                                    st[:], lhsT=xg[:, col * P:(col + 1) * P],
                                    rhs=o[:],
                                    start=(ti == 0),
                                    stop=(ti == len(cols) - 1))
                            p2 = pq.tile([P, SB], F32, name="p2", space="PSUM")
                            if cols:
                                sn = sp.tile([P, SB], BF16, name="sn")
                                nc.scalar.activation(
                                    sn[:], st[:],
                                    mybir.ActivationFunctionType.Copy)
                                nc.tensor.matmul(p2[:], lhsT=Bs[:], rhs=sn[:],
                                                 start=True, stop=False)
                                nc.tensor.matmul(
                                    p2[:], lhsT=Rs[:],
                                    rhs=xtbg[:, si * SB:(si + 1) * SB],
                                    start=False, stop=True)
                            else:
                                nc.tensor.matmul(
                                    p2[:], lhsT=Rs[:],
                                    rhs=xtbg[:, si * SB:(si + 1) * SB],
                                    start=True, stop=True)
                            nc.scalar.activation(
                                outg[:, si * SB:(si + 1) * SB], p2[:],
                                mybir.ActivationFunctionType.Relu,
                                bias=bs[:, :1])
                            if l < 3:
                                for h in range(2):
                                    c0 = si * SB + h * P
                                    tp2 = pt.tile([P, P], BF16, name="tp2",
                                                  space="PSUM")
                                    nc.tensor.transpose(
                                        tp2[:], outg[:, c0:c0 + P], ident[:])
                                    nc.scalar.activation(
                                        rowg[:, c0:c0 + P], tp2[:],
                                        mybir.ActivationFunctionType.Copy)
                        nc.sync.dma_start(
                            xTn[:, sbs[0] * SB:(sbs[0] + gn) * SB],
                            outg[:, :gn * SB])
                        if l < 3:
                            nc.sync.dma_start(
                                ag_in[l][sbs[0] * SB:(sbs[0] + gn) * SB, :]
                                .rearrange("(t p) d -> p t d", p=P),
                                rowg[:, :gn * SB].rearrange(
                                    "p (t d) -> p t d", d=D))
                        if l == 3 and sbs[0] < NSB_VAR:
                            gnv = sum(1 for s in sbs if s < NSB_VAR)
                            xtls = []
                            for hl in range(3):
                                xtl = og.tile([P, GSB * SB], BF16,
                                              name=f"xtl{hl}")
                                nc.sync.dma_start(
                                    xtl[:, :gnv * SB],
                                    xT_own[hl][:, sbs[0] * SB:
                                               (sbs[0] + gnv) * SB])
                                xtls.append(xtl)
                            ybuf = og.tile([P, 2 * GSB], F32, name="ybuf")
                            for si in range(gnv):
                                for h in range(2):
                                    cc0 = si * SB + h * P
                                    hp = pt.tile([P, P], F32, name="hp",
                                                 space="PSUM")
                                    for hl in range(3):
                                        nc.tensor.matmul(
                                            hp[:], lhsT=fc1c[hl][:],
                                            rhs=xtls[hl][:, cc0:cc0 + P],
                                            start=(hl == 0), stop=False)
                                    nc.tensor.matmul(
                                        hp[:], lhsT=fc1c[3][:],
                                        rhs=outg[:, cc0:cc0 + P],
                                        start=False, stop=True)
                                    hr = sp.tile([P, P], BF16, name="hr")
                                    nc.scalar.activation(
                                        hr[:], hp[:],
                                        mybir.ActivationFunctionType.Relu,
                                        bias=fb1[:, :1])
                                    yp = py.tile([P, 1], F32, name="yp",
                                                 space="PSUM")
                                    nc.tensor.matmul(yp[:], lhsT=hr[:],
                                                     rhs=f4w[:],
                                                     start=True, stop=True)
                                    nc.vector.tensor_scalar(
                                        ybuf[:, si * 2 + h:si * 2 + h + 1],
                                        yp[:], f4b[:, :1], None,
                                        op0=mybir.AluOpType.add)
                            nc.sync.dma_start(
                                y_out[sbs[0] * SB:(sbs[0] + gnv) * SB]
                                .rearrange("(g p) -> p g", p=P),
                                ybuf[:, :gnv * 2])
                if l < 3:
                    nc.gpsimd.collective_compute(
                        "AllGather", mybir.AluOpType.bypass,
                        replica_groups=rgroups,
                        ins=[ag_in[l][:]], outs=[x_full[l][:]])

    nc.compile()
    return nc


_CACHE = {}


def kernel(**inputs) -> np.ndarray:
    from concourse import bass_utils
    in_maps, tpb2, groups, colmap, call_tiles, T2, key = _preprocess(inputs)
    if key not in _CACHE:
        _CACHE[key] = _build_program(tpb2, groups, colmap, call_tiles, T2)
    nc = _CACHE[key]
    res = bass_utils.run_bass_kernel_spmd(
        nc, in_maps, core_ids=list(range(N_CORES)))
    ys = [res.results[c]["y_out"][:NV_CORE] for c in range(N_CORES)]
    return np.concatenate(ys, axis=0).astype(np.float32)
